# revision 1
# baseline (speedup 1.0000x reference)
"""TRN2 Bass kernel for fused MHA (softmax-over-query quirk) + out-proj + residual + LayerNorm.

Problem shapes (hardcoded): tokens [4,2048,1024], Wq/Wk [16,1024,64], Wv [16,1024,64],
Wo [1024,1024], gamma/beta [1024]. Output [4,2048,1024] fp32.

Sharding: 8 cores, core c owns (batch b=c//2, S-half jc=c%2) of the OUTPUT rows.
No collectives. Each core computes, for its batch b:
  qT[dk,i] (full S), kT[dk,j] (its half), V[i,dv] (full S) in bf16,
  scores^T[i,j] = q_i.k_j (PSUM fp32), e = exp(scores/8) (bf16),
  heads^T[dv,j] + rowsum row via a ones-column appended to V,
  multi^T = heads^T / rowsum, out = multi @ Wo + tokens, LayerNorm rows.

Math done in bf16 matmuls with fp32 PSUM accumulation; residual + LN in fp32.
Validated against fp32 reference: max abs err ~2e-4 (output absmax ~5).
"""

import numpy as np
import ml_dtypes

BF16 = ml_dtypes.bfloat16

B, S, D, H, DK, DV = 4, 2048, 1024, 16, 64, 64
NCORES = 8
NPAIR = 8     # head pairs
NKC = 8       # D // 128 contraction chunks
NIC = 16      # S // 128 i-chunks
JW = 1024     # j columns per core (S/2)
NJCH = 8      # JW // 128
LN_EPS = 1e-5

_CACHE = {}


def _build_nc(apply_affine):
    import concourse.tile as tile
    from concourse import bacc, mybir

    F32 = mybir.dt.float32
    BF = mybir.dt.bfloat16
    Exp = mybir.ActivationFunctionType.Exp
    Square = mybir.ActivationFunctionType.Square
    Sqrt = mybir.ActivationFunctionType.Sqrt
    mult = mybir.AluOpType.mult
    add = mybir.AluOpType.add
    AX = mybir.AxisListType.X

    nc = bacc.Bacc(
        "TRN2",
        target_bir_lowering=False,
        debug=False,
        enable_asserts=False,
        num_devices=NCORES,
    )

    # DRAM I/O (per-core views; host prepares layouts)
    tokT_d = nc.dram_tensor("tokT", (128, NKC, S), BF, kind="ExternalInput").ap()
    tokTj_d = nc.dram_tensor("tokTj", (128, NKC, JW), BF, kind="ExternalInput").ap()
    wq_d = nc.dram_tensor("wq", (128, NKC, H * DK), BF, kind="ExternalInput").ap()
    wk_d = nc.dram_tensor("wk", (128, NKC, H * DK), BF, kind="ExternalInput").ap()
    wv_d = nc.dram_tensor("wv", (128, NKC, H * DV), BF, kind="ExternalInput").ap()
    wo_d = nc.dram_tensor("wo", (128, NKC, D), BF, kind="ExternalInput").ap()
    tokres_d = nc.dram_tensor("tokres", (128, NJCH, D), F32, kind="ExternalInput").ap()
    if apply_affine:
        gamma_d = nc.dram_tensor("gamma_bc", (128, D), F32, kind="ExternalInput").ap()
        beta_d = nc.dram_tensor("beta_bc", (128, D), F32, kind="ExternalInput").ap()
    out_d = nc.dram_tensor("out", (128, NJCH, D), F32, kind="ExternalOutput").ap()
    from contextlib import ExitStack

    from concourse.bass import _add_dep_helper

    # Chain all PE matmuls in emission order: keeps row-tiled score pairs
    # adjacent (hardware runs them concurrently) and stops the scheduler from
    # interleaving row-conflicting matmuls between them.
    _prev_mm = [None]

    def mm(*args, **kwargs):
        inst = nc.tensor.matmul(*args, **kwargs)
        if _prev_mm[0] is not None:
            _add_dep_helper(inst.ins, _prev_mm[0].ins, sync=False, reason="pe-order")
        _prev_mm[0] = inst
        return inst

    with tile.TileContext(nc) as tc, ExitStack() as stack:
        persist = stack.enter_context(tc.tile_pool(name="persist", bufs=1))
        qT_sb = persist.tile([128, NPAIR, S], BF)          # [pair-dk, pr, i]
        kT_sb = persist.tile([128, NPAIR, JW], BF)         # [pair-dk, pr, j]
        v_sb = persist.tile([128, NIC, H, DV + 1], BF)     # [i%128, ic, h, dv|ones]
        # multi^T stored as one tile per 128-row chunk so out-proj dep tracking
        # stays per-pair (a single big tile serializes on the last DMA write)
        multiT = [
            persist.tile([128, JW], BF, name=f"multiT{kc}") for kc in range(NKC)
        ]
        eps_sb = persist.tile([128, 1], F32)
        if apply_affine:
            gamma_sb = persist.tile([128, D], F32)
            beta_sb = persist.tile([128, D], F32)
            nc.sync.dma_start(gamma_sb[:], gamma_d[:])
            nc.sync.dma_start(beta_sb[:], beta_d[:])
        nc.vector.memset(eps_sb[:], LN_EPS)
        for ic in range(NIC):
            nc.vector.memset(v_sb[:, ic, :, DV : DV + 1], 1.0)

        # pools that outlive pa/pw must be allocated first (LIFO release)
        psS = tc.alloc_tile_pool(name="psS", bufs=2, space="PSUM")
        psAcc = tc.alloc_tile_pool(name="psAcc", bufs=2, space="PSUM")
        pe_pool = stack.enter_context(tc.tile_pool(name="pe", bufs=4))
        pn_pool = stack.enter_context(tc.tile_pool(name="pn", bufs=2))
        pdram = stack.enter_context(tc.tile_pool(name="pdram", bufs=2, space="DRAM"))

        pa = tc.alloc_tile_pool(name="pa", bufs=1)
        tokT_sb = pa.tile([128, NKC, S], BF)
        tokTj_sb = pa.tile([128, NKC, JW], BF)
        wv_sb = pa.tile([128, NKC, H * DV], BF)
        # Per-pair Wq/Wk slices come through a small rotating pool (the pool
        # slot count gives ~1.5 pairs of DMA prefetch depth).
        pw = tc.alloc_tile_pool(name="pw", bufs=2)

        def load_w(pr, which):
            w_d = wq_d if which == "q" else wk_d
            t = pw.tile([128, NKC, 128], BF, tag="w", name=f"w{which}{pr}")
            nc.sync.dma_start(t[:], w_d[:, :, pr * 128 : (pr + 1) * 128])
            return t

        # DMA order: small weight slices for pairs 0/1 first, then tokens
        # (chunk-interleaved) so the first projection chains start ~3us in.
        w01 = {(pr, w): load_w(pr, w) for pr in (0, 1) for w in ("q", "k")}
        for kc in range(NKC):
            nc.sync.dma_start(tokT_sb[:, kc], tokT_d[:, kc])
        for kc in range(NKC):
            nc.sync.dma_start(tokTj_sb[:, kc], tokTj_d[:, kc])
        for kc in range(NKC):
            nc.sync.dma_start(wv_sb[:, kc], wv_d[:, kc])

        def proj_chain(w_t, pr, which, t):
            """One 512-wide projection chain via a borrowed scores-pool slot."""
            dst, rhs_sb = (qT_sb, tokT_sb) if which == "q" else (kT_sb, tokTj_sb)
            ps = psS.tile([128, 512], F32, tag="sc", name=f"pj{which}{pr}_{t}")
            for kc in range(NKC):
                mm(
                    ps[:],
                    w_t[:, kc, :],
                    rhs_sb[:, kc, t * 512 : (t + 1) * 512],
                    start=(kc == 0),
                    stop=(kc == NKC - 1),
                )
            nc.vector.tensor_copy(out=dst[:, pr, t * 512 : (t + 1) * 512], in_=ps[:])

        def proj_qkt(pr, which, w_t=None):
            if w_t is None:
                w_t = load_w(pr, which)
            for t in range(4 if which == "q" else 2):
                proj_chain(w_t, pr, which, t)

        def proj_v(ic):
            """V projection for one i-chunk via a borrowed scores-pool slot."""
            ps = psS.tile([128, 1024], F32, tag="sc", name=f"pjv{ic}")
            for kc in range(NKC):
                for nb in range(2):
                    mm(
                        ps[:, nb * 512 : (nb + 1) * 512],
                        tokT_sb[:, kc, ic * 128 : (ic + 1) * 128],
                        wv_sb[:, kc, nb * 512 : (nb + 1) * 512],
                        start=(kc == 0),
                        stop=(kc == NKC - 1),
                    )
            nc.vector.tensor_copy(
                out=v_sb[:, ic, :, 0:DV], in_=ps.rearrange("p (h v) -> p h v", h=H)
            )

        def normalize(pr, acc):
            """multi^T[h] = heads^T / rowsum; runs on DVE/DMA only."""
            for hh in range(2):
                h = 2 * pr + hh
                hraw = pn_pool.tile([DV + 1, JW], F32, tag="hraw", name=f"hraw{h}")
                nc.vector.tensor_copy(out=hraw[:], in_=acc[hh][:])  # frees acc
                rs_dram = pdram.tile([1, JW], F32, tag="rsd", name=f"rsd{h}")
                nc.sync.dma_start(out=rs_dram[:], in_=hraw[DV : DV + 1, :])
                rec_in = pn_pool.tile([DV, JW], F32, tag="rin", name=f"rin{h}")
                nc.gpsimd.dma_start(out=rec_in[:], in_=rs_dram.to_broadcast((DV, JW)))
                nc.vector.reciprocal_approx_fast(out=rec_in[:], in_=rec_in[:])
                if hh == 0:
                    nc.vector.tensor_tensor(
                        multiT[h // 2][0:64, :], hraw[0:DV, :], rec_in[:], mult
                    )
                else:
                    tmp64 = pn_pool.tile([DV, JW], BF, tag="tmp64", name=f"tmp{h}")
                    nc.vector.tensor_tensor(tmp64[:], hraw[0:DV, :], rec_in[:], mult)
                    nc.sync.dma_start(out=multiT[h // 2][64:128, :], in_=tmp64[:])

        def attention(hooks_by_pr, after_pair=None):
            """All pairs, flat: attnV lags one i-chunk behind scores/exp and
            crosses pair boundaries so the exp stream never waits on the PE."""
            pending_av = None   # (eTs, ic, pr, acc)
            finished = None     # (pr, acc) awaiting normalize
            acc_by_pr = {}
            for pr in range(NPAIR):
                acc_by_pr[pr] = [
                    psAcc.tile([DV + 1, JW], F32, tag="acc", name=f"acc{pr}_{hh}")
                    for hh in range(2)
                ]
                hooks = hooks_by_pr.get(pr, {})
                for ic in range(NIC):
                    ps_s = [
                        psS.tile([128, JW], F32, tag="sc", name=f"ps_s{pr}_{ic}_{hh}")
                        for hh in range(2)
                    ]
                    # scores^T, row-tiled pair (K=64 at partitions 0/64)
                    for hh in range(2):
                        for jb in range(2):
                            mm(
                                ps_s[hh][:, jb * 512 : (jb + 1) * 512],
                                qT_sb[hh * 64 : (hh + 1) * 64, pr, ic * 128 : (ic + 1) * 128],
                                kT_sb[hh * 64 : (hh + 1) * 64, pr, jb * 512 : (jb + 1) * 512],
                                start=True,
                                stop=True,
                            )
                    eTs = []
                    for hh in range(2):
                        eT = pe_pool.tile([128, JW], BF, tag="eT", name=f"eT{pr}_{ic}_{hh}")
                        nc.scalar.activation(eT[:], ps_s[hh][:], Exp, scale=0.125)
                        eTs.append(eT)
                    if ic in hooks:
                        hooks[ic]()
                    if pending_av is not None:
                        peT, pic, ppr, pacc = pending_av
                        for hh in range(2):
                            for jb in range(2):
                                mm(
                                    pacc[hh][:, jb * 512 : (jb + 1) * 512],
                                    v_sb[:, pic, 2 * ppr + hh, :],
                                    peT[hh][:, jb * 512 : (jb + 1) * 512],
                                    start=(pic == 0),
                                    stop=(pic == NIC - 1),
                                )
                        if pic == NIC - 1:
                            finished = (ppr, pacc)
                    if finished is not None:
                        normalize(*finished)
                        finished = None
                    pending_av = (eTs, ic, pr, acc_by_pr[pr])
                if after_pair and pr in after_pair:
                    after_pair[pr]()
            peT, pic, ppr, pacc = pending_av
            for hh in range(2):
                for jb in range(2):
                    mm(
                        pacc[hh][:, jb * 512 : (jb + 1) * 512],
                        v_sb[:, pic, 2 * ppr + hh, :],
                        peT[hh][:, jb * 512 : (jb + 1) * 512],
                        start=(pic == 0),
                        stop=(pic == NIC - 1),
                    )
            normalize(ppr, pacc)

        # upfront: qT/kT for all pairs (rotating weight tiles, DMA-prefetched);
        # the V projection rides inside pair 0's ACT-idle window.
        for pr in range(NPAIR):
            proj_qkt(pr, "q", w_t=w01.get((pr, "q")))
            proj_qkt(pr, "k", w_t=w01.get((pr, "k")))
        pw.release()

        pc_tiles = {}

        def open_phase_c():
            # pa's tensors are dead after pair 0's V hooks; reuse the space
            # for phase C inputs so their DMA overlaps pairs 1-7.
            pa.release()
            pc = stack.enter_context(tc.tile_pool(name="pc", bufs=1))
            pc_tiles["wo"] = pc.tile([128, NKC, D], BF, name="wo_sb")
            pc_tiles["tokres"] = pc.tile([128, NJCH, D], F32, name="tokres_sb")
            nc.sync.dma_start(pc_tiles["wo"][:], wo_d[:])
            nc.sync.dma_start(pc_tiles["tokres"][:], tokres_d[:])

        attention(
            {0: {ic: (lambda ic=ic: proj_v(ic)) for ic in range(NIC)}},
            after_pair={0: open_phase_c},
        )
        wo_sb = pc_tiles["wo"]
        tokres_sb = pc_tiles["tokres"]
        psAcc.release()
        psS.release()
        # ---------------- Phase C: out-proj + residual + LayerNorm ----------------
        with (
            tc.tile_pool(name="pC", bufs=2) as pC,
            tc.tile_pool(name="pStats", bufs=8) as pStats,
            tc.tile_pool(name="psC", bufs=4, space="PSUM") as psC,
        ):
            for jch in range(NJCH):
                ps_o = psC.tile([128, D], F32, tag="po", name=f"ps_o{jch}")
                for kc in range(NKC):
                    lhsT = multiT[kc][:, jch * 128 : (jch + 1) * 128]
                    for nb in range(2):
                        mm(
                            ps_o[:, nb * 512 : (nb + 1) * 512],
                            lhsT,
                            wo_sb[:, kc, nb * 512 : (nb + 1) * 512],
                            start=(kc == 0),
                            stop=(kc == NKC - 1),
                        )
                # x = psum + residual, sum_t = rowsum(x), in one DVE pass
                x_sb = pC.tile([128, D], F32, tag="x", name=f"x{jch}")
                sum_t = pStats.tile([128, 1], F32, tag="sum", name=f"sum{jch}")
                nc.vector.scalar_tensor_tensor(
                    out=x_sb[:],
                    in0=ps_o[:],
                    scalar=0.0,
                    in1=tokres_sb[:, jch, :],
                    op0=add,
                    op1=add,
                    accum_out=sum_t[:],
                )
                negmean = pStats.tile([128, 1], F32, tag="nm", name=f"nm{jch}")
                nc.vector.tensor_scalar_mul(negmean[:], sum_t[:], -1.0 / D)
                # ssq = sum((x-m)^2) on the otherwise-idle Scalar engine:
                # Square(x*1 + negmean) with accum_out
                scrap = pC.tile([128, D], BF, tag="scrap", name=f"scrap{jch}")
                ssq = pStats.tile([128, 1], F32, tag="ssq", name=f"ssq{jch}")
                nc.scalar.activation(
                    scrap[:], x_sb[:], Square, bias=negmean[:], accum_out=ssq[:]
                )
                std_t = pStats.tile([128, 1], F32, tag="std", name=f"std{jch}")
                nc.scalar.activation(std_t[:], ssq[:], Sqrt, bias=eps_sb[:], scale=1.0 / D)
                rstd = pStats.tile([128, 1], F32, tag="rstd", name=f"rstd{jch}")
                nc.vector.reciprocal(rstd[:], std_t[:])
                # (x - m) * rstd == x*rstd + (negmean*rstd), one ACT op
                rstd_nm = pStats.tile([128, 1], F32, tag="rnm", name=f"rnm{jch}")
                nc.vector.tensor_tensor(rstd_nm[:], negmean[:], rstd[:], mult)
                out_sb = pC.tile([128, D], F32, tag="out", name=f"out{jch}")
                nc.scalar.activation(
                    out_sb[:],
                    x_sb[:],
                    mybir.ActivationFunctionType.Identity,
                    bias=rstd_nm[:],
                    scale=rstd[:],
                )
                if apply_affine:
                    nc.gpsimd.tensor_tensor(out_sb[:], out_sb[:], gamma_sb[:], mult)
                    nc.gpsimd.tensor_tensor(out_sb[:], out_sb[:], beta_sb[:], add)
                nc.sync.dma_start(out_d[:, jch], out_sb[:])

    nc.compile()
    return nc


def _prep_inputs(tokens, Wq, Wk, Wv, Wo, gamma, beta):
    """Host-side layout prep. Returns per-core input maps."""
    tokens = np.ascontiguousarray(np.asarray(tokens, dtype=np.float32))
    # weights -> [p, kc, n] with row index kc*128+p
    def rows128(a):  # [1024, N] -> [128, 8, N]
        return np.ascontiguousarray(
            a.reshape(NKC, 128, a.shape[-1]).transpose(1, 0, 2)
        )

    wq_all = rows128(np.asarray(Wq).transpose(1, 0, 2).reshape(D, H * DK).astype(BF16))
    wk_all = rows128(np.asarray(Wk).transpose(1, 0, 2).reshape(D, H * DK).astype(BF16))
    wv_all = rows128(np.asarray(Wv).transpose(1, 0, 2).reshape(D, H * DV).astype(BF16))
    wo_all = rows128(np.asarray(Wo).astype(BF16))
    gamma_bc = np.ascontiguousarray(
        np.broadcast_to(np.asarray(gamma, np.float32), (128, D))
    )
    beta_bc = np.ascontiguousarray(
        np.broadcast_to(np.asarray(beta, np.float32), (128, D))
    )

    tokT_by_b = []
    for b in range(B):
        tokT_by_b.append(rows128(tokens[b].T.astype(BF16)))  # [128, 8, 2048]

    in_maps = []
    for c in range(NCORES):
        b, jc = c // 2, c % 2
        tokT = tokT_by_b[b]
        tokTj = np.ascontiguousarray(tokT[:, :, jc * JW : (jc + 1) * JW])
        tokres = np.ascontiguousarray(
            tokens[b, jc * JW : (jc + 1) * JW]
            .reshape(NJCH, 128, D)
            .transpose(1, 0, 2)
        )
        in_maps.append(
            {
                "tokT": tokT,
                "tokTj": tokTj,
                "wq": wq_all,
                "wk": wk_all,
                "wv": wv_all,
                "wo": wo_all,
                "tokres": tokres,
                "gamma_bc": gamma_bc,
                "beta_bc": beta_bc,
            }
        )
    return in_maps


def run(inputs, trace=False, tmpdir=None):
    """Run on hardware; returns (output, BassKernelResults)."""
    from concourse.bass_utils import run_bass_kernel_spmd

    apply_affine = not (
        np.all(np.asarray(inputs["gamma"]) == 1.0)
        and np.all(np.asarray(inputs["beta"]) == 0.0)
    )
    key = ("nc", apply_affine)
    if key not in _CACHE:
        _CACHE[key] = _build_nc(apply_affine)
    nc = _CACHE[key]
    in_maps = _prep_inputs(**inputs)
    res = run_bass_kernel_spmd(
        nc, in_maps, core_ids=list(range(NCORES)), trace=trace, tmpdir=tmpdir
    )
    out = np.empty((B, S, D), np.float32)
    for c in range(NCORES):
        b, jc = c // 2, c % 2
        o = res.results[c]["out"]  # [128, 8, 1024]
        out[b, jc * JW : (jc + 1) * JW] = (
            o.transpose(1, 0, 2).reshape(JW, D)
        )
    return out, res


def kernel(tokens, Wq, Wk, Wv, Wo, gamma, beta):
    out, _ = run(
        dict(tokens=tokens, Wq=Wq, Wk=Wk, Wv=Wv, Wo=Wo, gamma=gamma, beta=beta)
    )
    return out



# revision 17
# speedup vs baseline: 1.1414x; 1.1414x over previous
"""TRN2 Bass kernel for fused MHA (softmax-over-query quirk) + out-proj + residual + LayerNorm.

Problem shapes (hardcoded): tokens [4,2048,1024], Wq/Wk [16,1024,64], Wv [16,1024,64],
Wo [1024,1024], gamma/beta [1024]. Output [4,2048,1024] fp32.

Sharding: 8 cores, core c owns (batch b=c//2, S-half jc=c%2) of the OUTPUT rows.
No collectives.

v2: fp8 (float8e4, DoubleRow perf mode) for all projections, attnV and out-proj
matmuls (weights pre-scaled x32 on host; scales folded back into the exp scale
and the phase-C residual add). Scores stay bf16 with row-tiled head pairs.
exp is split across two engines: ACT computes real Exp (fp8 out); DVE computes
a Schraudolph bit-trick exp (y=a*x+b -> int8 convert == fp8e4 bits). The
Schraudolph path has ~7% per-element error, but the attention path contributes
~1% of the output (residual dominates), so the end-to-end error stays ~1e-3.
Projections for pairs 2..7 and the V projection are interleaved into the
attention loop of earlier pairs so the exp engines start at t~=10us.
"""

import numpy as np
import ml_dtypes

BF16 = ml_dtypes.bfloat16
F8 = ml_dtypes.float8_e4m3

B, S, D, H, DK, DV = 4, 2048, 1024, 16, 64, 64
NCORES = 8
NPAIR = 8     # head pairs
NKC = 8       # D // 128 contraction chunks
NIC = 16      # S // 128 i-chunks
JW = 1024     # j columns per core (S/2)
NJCH = 8      # JW // 128
LN_EPS = 1e-5

WS = 32.0                      # host-side weight scale (wq/wk/wv/wo)
SCALE_EXP = 0.125 / (WS * WS)  # exp(ps * SCALE_EXP) == exp(s_true / 8)
A_SCH = 11.541560327111707 * SCALE_EXP   # (8/ln2) * SCALE_EXP
B_SCH = 55.63                            # fp8e4 (bias 7) Schraudolph offset
CINV = 1.0 / (64.0 * WS)       # undo multi(x64) * wo(x32) scale in phase C

_CACHE = {}


def _build_nc(apply_affine):
    import concourse.tile as tile
    from concourse import bacc, mybir

    F32 = mybir.dt.float32
    BF = mybir.dt.bfloat16
    FP8 = mybir.dt.float8e4
    I8 = mybir.dt.int8
    DR = mybir.MatmulPerfMode.DoubleRow
    Exp = mybir.ActivationFunctionType.Exp
    Square = mybir.ActivationFunctionType.Square
    Sqrt = mybir.ActivationFunctionType.Sqrt
    mult = mybir.AluOpType.mult
    add = mybir.AluOpType.add

    nc = bacc.Bacc(
        "TRN2",
        target_bir_lowering=False,
        debug=False,
        enable_asserts=False,
        num_devices=NCORES,
    )

    # DRAM I/O (per-core views; host prepares layouts)
    tokT_d = nc.dram_tensor("tokT", (128, NKC, S), FP8, kind="ExternalInput").ap()
    tokTj_d = nc.dram_tensor("tokTj", (128, NKC, JW), FP8, kind="ExternalInput").ap()
    wq_d = nc.dram_tensor("wq", (128, NKC, H * DK), FP8, kind="ExternalInput").ap()
    wk_d = nc.dram_tensor("wk", (128, NKC, H * DK), FP8, kind="ExternalInput").ap()
    wv_d = nc.dram_tensor("wv", (128, NKC, H * DV), FP8, kind="ExternalInput").ap()
    wo_d = nc.dram_tensor("wo", (128, NKC, D), FP8, kind="ExternalInput").ap()
    tokres_d = nc.dram_tensor("tokres", (128, NJCH, D), F32, kind="ExternalInput").ap()
    if apply_affine:
        gamma_d = nc.dram_tensor("gamma_bc", (128, D), F32, kind="ExternalInput").ap()
        beta_d = nc.dram_tensor("beta_bc", (128, D), F32, kind="ExternalInput").ap()
    out_d = nc.dram_tensor("out", (128, NJCH, D), F32, kind="ExternalOutput").ap()
    from contextlib import ExitStack

    from concourse.bass import _add_dep_helper

    # Chain all PE matmuls in emission order: keeps row-tiled score pairs
    # adjacent (hardware runs them concurrently) and stops the scheduler from
    # interleaving row-conflicting matmuls between them.
    _prev_mm = [None]

    def mm(*args, **kwargs):
        inst = nc.tensor.matmul(*args, **kwargs)
        if _prev_mm[0] is not None:
            _add_dep_helper(inst.ins, _prev_mm[0].ins, sync=False, reason="pe-order")
        _prev_mm[0] = inst
        return inst

    with tile.TileContext(nc) as tc, ExitStack() as stack:
        persist = stack.enter_context(tc.tile_pool(name="persist", bufs=1))
        qT_sb = persist.tile([128, NPAIR, S], BF)           # [pair-dk, pr, i]
        kT_sb = persist.tile([128, NPAIR, JW], BF)          # [pair-dk, pr, j]
        v_sb = persist.tile([128, NIC, H, DV + 1], FP8)     # [i%128, ic, h, dv|0.5]
        # multi^T (x64) in fp8, kc-paired for DoubleRow out-proj
        multiT = [
            persist.tile([128, 2, JW], FP8, name=f"multiT{t}") for t in range(4)
        ]
        tokT_sb = persist.tile([128, NKC, S], FP8)
        tokTj_sb = persist.tile([128, NKC, JW], FP8)
        wv_sb = persist.tile([128, NKC, H * DV], FP8)
        wo_sb = persist.tile([128, NKC, D], FP8)
        tokres_sb = persist.tile([128, NJCH, D], F32)
        eps_sb = persist.tile([128, 1], F32)
        if apply_affine:
            gamma_sb = persist.tile([128, D], F32)
            beta_sb = persist.tile([128, D], F32)
            nc.sync.dma_start(gamma_sb[:], gamma_d[:])
            nc.sync.dma_start(beta_sb[:], beta_d[:])
        nc.vector.memset(eps_sb[:], LN_EPS)
        for ic in range(NIC):
            # 0.5 ones-column => rowsum row accumulates 0.5*sum(e); its
            # reciprocal is then 2/sum, which bakes in the x64 multi scale
            # together with the x32 of v.
            nc.vector.memset(v_sb[:, ic, :, DV : DV + 1], 0.5)

        psS = tc.alloc_tile_pool(name="psS", bufs=2, space="PSUM")
        psAcc = tc.alloc_tile_pool(name="psAcc", bufs=2, space="PSUM")
        pe_pool = stack.enter_context(tc.tile_pool(name="pe", bufs=3))
        pn_pool = stack.enter_context(tc.tile_pool(name="pn", bufs=2))
        pdram = stack.enter_context(tc.tile_pool(name="pdram", bufs=2, space="DRAM"))
        pw = stack.enter_context(tc.tile_pool(name="pw", bufs=6))

        def load_w(pr, which):
            w_d = wq_d if which == "q" else wk_d
            t = pw.tile([128, NKC, 128], FP8, tag="w", name=f"w{which}{pr}")
            nc.sync.dma_start(t[:], w_d[:, :, pr * 128 : (pr + 1) * 128])
            return t

        # DMA order: first tokT chunks + pair-0/1 weight slices (sync queue),
        # with tokTj/wv in parallel on the gpsimd queue, so the first
        # projection chains and pair-0 scores start as early as possible.
        for kc in (0, 1):
            nc.sync.dma_start(tokT_sb[:, kc], tokT_d[:, kc])
        wslices = {(pr, w): load_w(pr, w) for pr in (0, 1) for w in ("q", "k")}
        for kc in range(2, NKC):
            nc.sync.dma_start(tokT_sb[:, kc], tokT_d[:, kc])
        for kc in range(NKC):
            nc.sync.dma_start(tokTj_sb[:, kc], tokTj_d[:, kc])
        for kc in range(NKC):
            nc.sync.dma_start(wv_sb[:, kc], wv_d[:, kc])

        def proj_chain(w_t, pr, which, t):
            """One 512-wide fp8-DR projection chain via a scores-pool slot."""
            dst, rhs_sb = (qT_sb, tokT_sb) if which == "q" else (kT_sb, tokTj_sb)
            ps = psS.tile([128, 512], F32, tag="sc", name=f"pj{which}{pr}_{t}")
            for c in range(NKC // 2):
                mm(
                    ps[:],
                    w_t[:, 2 * c : 2 * c + 2, :],
                    rhs_sb[:, 2 * c : 2 * c + 2, t * 512 : (t + 1) * 512],
                    start=(c == 0),
                    stop=(c == NKC // 2 - 1),
                    perf_mode=DR,
                )
            nc.vector.tensor_copy(out=dst[:, pr, t * 512 : (t + 1) * 512], in_=ps[:])

        def proj_v(ic):
            """V projection (fp8 DR) for one i-chunk via a scores-pool slot."""
            ps = psS.tile([128, 1024], F32, tag="sc", name=f"pjv{ic}")
            for c in range(NKC // 2):
                for nb in range(2):
                    mm(
                        ps[:, nb * 512 : (nb + 1) * 512],
                        tokT_sb[:, 2 * c : 2 * c + 2, ic * 128 : (ic + 1) * 128],
                        wv_sb[:, 2 * c : 2 * c + 2, nb * 512 : (nb + 1) * 512],
                        start=(c == 0),
                        stop=(c == NKC // 2 - 1),
                        perf_mode=DR,
                    )
            nc.vector.tensor_copy(
                out=v_sb[:, ic, :, 0:DV], in_=ps.rearrange("p (h v) -> p h v", h=H)
            )

        def exp_engine(pr, hh):
            # ACT always takes hh0; it also takes hh1 for pairs 0-1 while DVE
            # is busy with projection copies.
            return "act" if (hh == 0 or pr < 2) else "dve"

        def normalize_hh(pr, hh, acc):
            """multiT (x64, fp8) = heads / rowsum for one head. The PSUM->SBUF
            staging copy runs on ACT (idle at pair boundaries); rest DVE/DMA."""
            t, ko = pr // 2, pr % 2
            Ident = mybir.ActivationFunctionType.Identity
            hraw = pn_pool.tile([DV + 1, JW], F32, tag="hraw", name=f"hraw{pr}_{hh}")
            nc.scalar.activation(hraw[:], acc[hh][:], Ident)  # frees acc
            rs_dram = pdram.tile([1, JW], F32, tag="rsd", name=f"rsd{pr}_{hh}")
            nc.sync.dma_start(out=rs_dram[:], in_=hraw[DV : DV + 1, :])
            rec_in = pn_pool.tile([DV, JW], F32, tag="rin", name=f"rin{pr}_{hh}")
            nc.gpsimd.dma_start(out=rec_in[:], in_=rs_dram.to_broadcast((DV, JW)))
            nc.vector.reciprocal_approx_fast(out=rec_in[:], in_=rec_in[:])
            if hh == 0:
                nc.vector.tensor_tensor(
                    multiT[t][0:DV, ko, :], hraw[0:DV, :], rec_in[:], mult
                )
            else:
                tmp64 = pn_pool.tile([DV, JW], FP8, tag="tmp64", name=f"tmp{pr}")
                nc.vector.tensor_tensor(tmp64[:], hraw[0:DV, :], rec_in[:], mult)
                nc.sync.dma_start(out=multiT[t][DV:128, ko, :], in_=tmp64[:])

        def attention(hooks_by_pr):
            """All pairs; attnV (fp8 DR over i-chunk pairs) lags the exp
            stream and is split by head: hh0 flushes on even steps, hh1 on
            odd steps, so every step has PE work covering the exp latency."""
            pending = None      # (pr, icp, eT, acc), flushed one hh at a time
            pending_hh = 0
            acc_by_pr = {}
            eT_cur = {}

            def flush_one():
                nonlocal pending, pending_hh
                if pending is None:
                    return
                ppr, picp, peT, pacc = pending
                hh = pending_hh
                for jb in range(2):
                    mm(
                        pacc[hh][:, jb * 512 : (jb + 1) * 512],
                        v_sb[:, 2 * picp : 2 * picp + 2, 2 * ppr + hh, :],
                        peT[hh][:, :, jb * 512 : (jb + 1) * 512],
                        start=(picp == 0),
                        stop=(picp == NIC // 2 - 1),
                        perf_mode=DR,
                    )
                if picp == NIC // 2 - 1:
                    normalize_hh(ppr, hh, pacc)
                if hh == 1:
                    pending = None
                pending_hh = 1 - pending_hh

            for pr in range(NPAIR):
                acc_by_pr[pr] = [
                    psAcc.tile([DV + 1, JW], F32, tag="acc", name=f"acc{pr}_{hh}")
                    for hh in range(2)
                ]
                hooks = hooks_by_pr.get(pr, {})
                for ic in range(NIC):
                    if ic % 2 == 0:
                        eT_cur = {
                            hh: pe_pool.tile(
                                [128, 2, JW], FP8, tag=f"eT{hh}",
                                name=f"eT{pr}_{ic // 2}_{hh}",
                            )
                            for hh in range(2)
                        }
                    ps_s = [
                        psS.tile([128, JW], F32, tag="sc", name=f"ps_s{pr}_{ic}_{hh}")
                        for hh in range(2)
                    ]
                    # scores^T, row-tiled pair (K=64 at partitions 0/64)
                    for hh in range(2):
                        for jb in range(2):
                            mm(
                                ps_s[hh][:, jb * 512 : (jb + 1) * 512],
                                qT_sb[hh * 64 : (hh + 1) * 64, pr, ic * 128 : (ic + 1) * 128],
                                kT_sb[hh * 64 : (hh + 1) * 64, pr, jb * 512 : (jb + 1) * 512],
                                start=True,
                                stop=True,
                            )
                    for hh in range(2):
                        slot = eT_cur[hh][:, ic % 2, :]
                        if exp_engine(pr, hh) == "act":
                            nc.scalar.activation(slot, ps_s[hh][:], Exp, scale=SCALE_EXP)
                        else:
                            nc.vector.tensor_scalar(
                                out=slot.bitcast(I8), in0=ps_s[hh][:],
                                scalar1=A_SCH, scalar2=B_SCH, op0=mult, op1=add,
                            )
                    flush_one()
                    if ic % 2 == 1:
                        assert pending is None
                        pending = (pr, (ic - 1) // 2, eT_cur, acc_by_pr[pr])
                    if ic in hooks:
                        for fn in hooks[ic]:
                            fn()
            flush_one()
            flush_one()

        # upfront: qT/kT for pairs 0-1 only; the rest rides inside the loop.
        for pr in (0, 1):
            for t in range(4):
                proj_chain(wslices[(pr, "q")], pr, "q", t)
            for t in range(2):
                proj_chain(wslices[(pr, "k")], pr, "k", t)

        hooks = {pr: {} for pr in range(NPAIR)}

        def add_hook(pr, ic, fn):
            hooks[pr].setdefault(ic, []).append(fn)

        # All hooks sit on odd steps, after the attnV flush, so their PSUM
        # slot (freed by an exp ~1 step earlier) never stalls the PE.
        # V projection: ALL 16 chunks inside pair 0, two per odd step —
        # chunk pair (2k, 2k+1) lands at step 2k+1, just ahead of the
        # attnV flush that consumes it at step 2k+2.
        for k in range(8):
            add_hook(0, 2 * k + 1, lambda k=k: (proj_v(2 * k), proj_v(2 * k + 1)))
        # Q/K projections: pairs 2-3 in pair 1 (doubled), pair p>=4 in pair
        # p-2 (one chain per odd step). Weight slices load one pair ahead.
        chains = [("q", 0), ("q", 1), ("q", 2), ("q", 3), ("k", 0), ("k", 1)]
        add_hook(0, 3, lambda: wslices.update({(2, "q"): load_w(2, "q"),
                                               (2, "k"): load_w(2, "k")}))
        add_hook(0, 9, lambda: wslices.update({(3, "q"): load_w(3, "q"),
                                               (3, "k"): load_w(3, "k")}))
        for i, (w, t) in enumerate(chains):
            add_hook(1, 2 * i + 1, lambda w=w, t=t: (
                proj_chain(wslices[(2, w)], 2, w, t),
                proj_chain(wslices[(3, w)], 3, w, t)))
        for p in range(4, NPAIR):
            host = p - 2
            add_hook(host - 1, 1, lambda p=p: wslices.update(
                {(p, "q"): load_w(p, "q"), (p, "k"): load_w(p, "k")}))
            for i, (w, t) in enumerate(chains):
                add_hook(host, 2 * i + 3, lambda p=p, w=w, t=t: proj_chain(
                    wslices[(p, w)], p, w, t))
        # phase C inputs arrive late so they don't compete with early DMAs
        def load_phase_c():
            nc.sync.dma_start(wo_sb[:], wo_d[:])
            nc.sync.dma_start(tokres_sb[:], tokres_d[:])
        add_hook(5, 14, load_phase_c)

        attention(hooks)
        psAcc.release()
        psS.release()

        # ---------------- Phase C: out-proj + residual + LayerNorm ----------------
        def outproj_chain(ps_o, jch, ts):
            for t in ts:
                lhsT = multiT[t][:, :, jch * 128 : (jch + 1) * 128]
                for nb in range(2):
                    mm(
                        ps_o[:, nb * 512 : (nb + 1) * 512],
                        lhsT,
                        wo_sb[:, 2 * t : 2 * t + 2, nb * 512 : (nb + 1) * 512],
                        start=(t == 0),
                        stop=(t == 3),
                        perf_mode=DR,
                    )

        with (
            tc.tile_pool(name="pC", bufs=2) as pC,
            tc.tile_pool(name="pStats", bufs=8) as pStats,
            tc.tile_pool(name="psC", bufs=2, space="PSUM") as psC,
        ):
            # jch 0-1 start their kc-pairs 0-2 (pairs 0-5, long since
            # normalized) while pair 6/7's normalize still runs.
            early = [
                psC.tile([128, D], F32, tag="po", name=f"ps_o{jch}")
                for jch in range(2)
            ]
            for jch in range(2):
                outproj_chain(early[jch], jch, (0, 1, 2))
            for jch in range(NJCH):
                if jch < 2:
                    ps_o = early[jch]
                    outproj_chain(ps_o, jch, (3,))
                else:
                    ps_o = psC.tile([128, D], F32, tag="po", name=f"ps_o{jch}")
                    outproj_chain(ps_o, jch, (0, 1, 2, 3))
                # x = psum/2048 + residual, sum_t = rowsum(x), in one DVE pass
                x_sb = pC.tile([128, D], F32, tag="x", name=f"x{jch}")
                sum_t = pStats.tile([128, 1], F32, tag="sum", name=f"sum{jch}")
                nc.vector.scalar_tensor_tensor(
                    out=x_sb[:],
                    in0=ps_o[:],
                    scalar=CINV,
                    in1=tokres_sb[:, jch, :],
                    op0=mult,
                    op1=add,
                    accum_out=sum_t[:],
                )
                negmean = pStats.tile([128, 1], F32, tag="nm", name=f"nm{jch}")
                nc.vector.tensor_scalar_mul(negmean[:], sum_t[:], -1.0 / D)
                # ssq = sum((x-m)^2) on the otherwise-idle Scalar engine
                scrap = pC.tile([128, D], BF, tag="scrap", name=f"scrap{jch}")
                ssq = pStats.tile([128, 1], F32, tag="ssq", name=f"ssq{jch}")
                nc.scalar.activation(
                    scrap[:], x_sb[:], Square, bias=negmean[:], accum_out=ssq[:]
                )
                std_t = pStats.tile([128, 1], F32, tag="std", name=f"std{jch}")
                nc.scalar.activation(std_t[:], ssq[:], Sqrt, bias=eps_sb[:], scale=1.0 / D)
                rstd = pStats.tile([128, 1], F32, tag="rstd", name=f"rstd{jch}")
                nc.vector.reciprocal(rstd[:], std_t[:])
                rstd_nm = pStats.tile([128, 1], F32, tag="rnm", name=f"rnm{jch}")
                nc.vector.tensor_tensor(rstd_nm[:], negmean[:], rstd[:], mult)
                out_sb = pC.tile([128, D], F32, tag="out", name=f"out{jch}")
                nc.scalar.activation(
                    out_sb[:],
                    x_sb[:],
                    mybir.ActivationFunctionType.Identity,
                    bias=rstd_nm[:],
                    scale=rstd[:],
                )
                if apply_affine:
                    nc.gpsimd.tensor_tensor(out_sb[:], out_sb[:], gamma_sb[:], mult)
                    nc.gpsimd.tensor_tensor(out_sb[:], out_sb[:], beta_sb[:], add)
                nc.sync.dma_start(out_d[:, jch], out_sb[:])

    nc.compile()
    return nc


def _prep_inputs(tokens, Wq, Wk, Wv, Wo, gamma, beta):
    """Host-side layout prep. Returns per-core input maps."""
    tokens = np.ascontiguousarray(np.asarray(tokens, dtype=np.float32))

    def rows128(a):  # [1024, N] -> [128, 8, N]
        return np.ascontiguousarray(
            a.reshape(NKC, 128, a.shape[-1]).transpose(1, 0, 2)
        )

    wq_all = rows128((np.asarray(Wq).transpose(1, 0, 2).reshape(D, H * DK) * WS).astype(F8))
    wk_all = rows128((np.asarray(Wk).transpose(1, 0, 2).reshape(D, H * DK) * WS).astype(F8))
    wv_all = rows128((np.asarray(Wv).transpose(1, 0, 2).reshape(D, H * DV) * WS).astype(F8))
    wo_all = rows128((np.asarray(Wo) * WS).astype(F8))
    gamma_bc = np.ascontiguousarray(
        np.broadcast_to(np.asarray(gamma, np.float32), (128, D))
    )
    beta_bc = np.ascontiguousarray(
        np.broadcast_to(np.asarray(beta, np.float32), (128, D))
    )

    tokT_by_b = []
    for b in range(B):
        tokT_by_b.append(rows128(tokens[b].T.astype(F8)))  # [128, 8, 2048] fp8

    in_maps = []
    for c in range(NCORES):
        b, jc = c // 2, c % 2
        tokT = tokT_by_b[b]
        tokTj = np.ascontiguousarray(tokT[:, :, jc * JW : (jc + 1) * JW])
        tokres = np.ascontiguousarray(
            tokens[b, jc * JW : (jc + 1) * JW]
            .reshape(NJCH, 128, D)
            .transpose(1, 0, 2)
        )
        in_maps.append(
            {
                "tokT": tokT,
                "tokTj": tokTj,
                "wq": wq_all,
                "wk": wk_all,
                "wv": wv_all,
                "wo": wo_all,
                "tokres": tokres,
                "gamma_bc": gamma_bc,
                "beta_bc": beta_bc,
            }
        )
    return in_maps


def run(inputs, trace=False, tmpdir=None):
    """Run on hardware; returns (output, BassKernelResults)."""
    from concourse.bass_utils import run_bass_kernel_spmd

    apply_affine = not (
        np.all(np.asarray(inputs["gamma"]) == 1.0)
        and np.all(np.asarray(inputs["beta"]) == 0.0)
    )
    key = ("nc", apply_affine)
    if key not in _CACHE:
        _CACHE[key] = _build_nc(apply_affine)
    nc = _CACHE[key]
    in_maps = _prep_inputs(**inputs)
    if not apply_affine:
        for m in in_maps:
            m.pop("gamma_bc"), m.pop("beta_bc")
    res = run_bass_kernel_spmd(
        nc, in_maps, core_ids=list(range(NCORES)), trace=trace, tmpdir=tmpdir
    )
    out = np.empty((B, S, D), np.float32)
    for c in range(NCORES):
        b, jc = c // 2, c % 2
        o = res.results[c]["out"]  # [128, 8, 1024]
        out[b, jc * JW : (jc + 1) * JW] = (
            o.transpose(1, 0, 2).reshape(JW, D)
        )
    return out, res


def kernel(tokens, Wq, Wk, Wv, Wo, gamma, beta):
    out, _ = run(
        dict(tokens=tokens, Wq=Wq, Wk=Wk, Wv=Wv, Wo=Wo, gamma=gamma, beta=beta)
    )
    return out


# revision 21
# speedup vs baseline: 1.1598x; 1.0161x over previous
"""TRN2 Bass kernel for fused MHA (softmax-over-query quirk) + out-proj + residual + LayerNorm.

Problem shapes (hardcoded): tokens [4,2048,1024], Wq/Wk [16,1024,64], Wv [16,1024,64],
Wo [1024,1024], gamma/beta [1024]. Output [4,2048,1024] fp32.

Sharding: 8 cores, core c owns (batch b=c//2, S-half jc=c%2) of the OUTPUT rows.
No collectives.

v2: fp8 (float8e4, DoubleRow perf mode) for all projections, attnV and out-proj
matmuls (weights pre-scaled x32 on host; scales folded back into the exp scale
and the phase-C residual add). Scores stay bf16 with row-tiled head pairs.
exp is split across two engines: ACT computes real Exp (fp8 out); DVE computes
a Schraudolph bit-trick exp (y=a*x+b -> int8 convert == fp8e4 bits). The
Schraudolph path has ~7% per-element error, but the attention path contributes
~1% of the output (residual dominates), so the end-to-end error stays ~1e-3.
Projections for pairs 2..7 and the V projection are interleaved into the
attention loop of earlier pairs so the exp engines start at t~=10us.
"""

import numpy as np
import ml_dtypes

BF16 = ml_dtypes.bfloat16
F8 = ml_dtypes.float8_e4m3

B, S, D, H, DK, DV = 4, 2048, 1024, 16, 64, 64
NCORES = 8
NPAIR = 8     # head pairs
NKC = 8       # D // 128 contraction chunks
NIC = 16      # S // 128 i-chunks
JW = 1024     # j columns per core (S/2)
NJCH = 8      # JW // 128
LN_EPS = 1e-5

WS = 32.0                      # host-side weight scale (wq/wk/wv/wo)
SCALE_EXP = 0.125 / (WS * WS)  # exp(ps * SCALE_EXP) == exp(s_true / 8)
A_SCH = 11.541560327111707 * SCALE_EXP   # (8/ln2) * SCALE_EXP
B_SCH = 55.63                            # fp8e4 (bias 7) Schraudolph offset
CINV = 1.0 / (64.0 * WS)       # undo multi(x64) * wo(x32) scale in phase C

_CACHE = {}


def _build_nc(apply_affine):
    import concourse.tile as tile
    from concourse import bacc, mybir

    F32 = mybir.dt.float32
    BF = mybir.dt.bfloat16
    FP8 = mybir.dt.float8e4
    I8 = mybir.dt.int8
    DR = mybir.MatmulPerfMode.DoubleRow
    Exp = mybir.ActivationFunctionType.Exp
    Square = mybir.ActivationFunctionType.Square
    Sqrt = mybir.ActivationFunctionType.Sqrt
    mult = mybir.AluOpType.mult
    add = mybir.AluOpType.add

    nc = bacc.Bacc(
        "TRN2",
        target_bir_lowering=False,
        debug=False,
        enable_asserts=False,
        num_devices=NCORES,
    )

    # DRAM I/O (per-core views; host prepares layouts)
    tokT_d = nc.dram_tensor("tokT", (128, NKC, S), FP8, kind="ExternalInput").ap()
    tokTj_d = nc.dram_tensor("tokTj", (128, NKC, JW), FP8, kind="ExternalInput").ap()
    wq_d = nc.dram_tensor("wq", (128, NKC, H * DK), FP8, kind="ExternalInput").ap()
    wk_d = nc.dram_tensor("wk", (128, NKC, H * DK), FP8, kind="ExternalInput").ap()
    wv_d = nc.dram_tensor("wv", (128, NKC, H * DV), FP8, kind="ExternalInput").ap()
    wo_d = nc.dram_tensor("wo", (128, NKC, D), FP8, kind="ExternalInput").ap()
    tokres_d = nc.dram_tensor("tokres", (128, NJCH, D), F32, kind="ExternalInput").ap()
    if apply_affine:
        gamma_d = nc.dram_tensor("gamma_bc", (128, D), F32, kind="ExternalInput").ap()
        beta_d = nc.dram_tensor("beta_bc", (128, D), F32, kind="ExternalInput").ap()
    out_d = nc.dram_tensor("out", (128, NJCH, D), F32, kind="ExternalOutput").ap()
    from contextlib import ExitStack

    from concourse.bass import _add_dep_helper

    # Chain all PE matmuls in emission order: keeps row-tiled score pairs
    # adjacent (hardware runs them concurrently) and stops the scheduler from
    # interleaving row-conflicting matmuls between them.
    _prev_mm = [None]

    def mm(*args, **kwargs):
        inst = nc.tensor.matmul(*args, **kwargs)
        if _prev_mm[0] is not None:
            _add_dep_helper(inst.ins, _prev_mm[0].ins, sync=False, reason="pe-order")
        _prev_mm[0] = inst
        return inst

    with tile.TileContext(nc) as tc, ExitStack() as stack:
        persist = stack.enter_context(tc.tile_pool(name="persist", bufs=1))
        qT_sb = persist.tile([128, NPAIR, S], BF)           # [pair-dk, pr, i]
        kT_sb = persist.tile([128, NPAIR, JW], BF)          # [pair-dk, pr, j]
        v_sb = persist.tile([128, NIC, H, DV + 1], FP8)     # [i%128, ic, h, dv|0.5]
        # multi^T (x64) in fp8, kc-paired for DoubleRow out-proj
        multiT = [
            persist.tile([128, 2, JW], FP8, name=f"multiT{t}") for t in range(4)
        ]
        tokT_sb = persist.tile([128, NKC, S], FP8)
        tokTj_sb = persist.tile([128, NKC, JW], FP8)
        wv_sb = persist.tile([128, NKC, H * DV], FP8)
        wo_sb = persist.tile([128, NKC, D], FP8)
        tokres_sb = persist.tile([128, NJCH, D], F32)
        eps_sb = persist.tile([128, 1], F32)
        if apply_affine:
            gamma_sb = persist.tile([128, D], F32)
            beta_sb = persist.tile([128, D], F32)
            nc.sync.dma_start(gamma_sb[:], gamma_d[:])
            nc.sync.dma_start(beta_sb[:], beta_d[:])
        nc.vector.memset(eps_sb[:], LN_EPS)
        for ic in range(NIC):
            # 0.5 ones-column => rowsum row accumulates 0.5*sum(e); its
            # reciprocal is then 2/sum, which bakes in the x64 multi scale
            # together with the x32 of v.
            nc.vector.memset(v_sb[:, ic, :, DV : DV + 1], 0.5)

        psS = tc.alloc_tile_pool(name="psS", bufs=2, space="PSUM")
        psAcc = tc.alloc_tile_pool(name="psAcc", bufs=2, space="PSUM")
        pe_pool = stack.enter_context(tc.tile_pool(name="pe", bufs=3))
        pn_pool = stack.enter_context(tc.tile_pool(name="pn", bufs=2))
        pdram = stack.enter_context(tc.tile_pool(name="pdram", bufs=2, space="DRAM"))
        pw = stack.enter_context(tc.tile_pool(name="pw", bufs=6))

        def load_w(pr, which):
            w_d = wq_d if which == "q" else wk_d
            t = pw.tile([128, NKC, 128], FP8, tag="w", name=f"w{which}{pr}")
            nc.sync.dma_start(t[:], w_d[:, :, pr * 128 : (pr + 1) * 128])
            return t

        # DMA order: first tokT chunks + pair-0/1 weight slices (sync queue),
        # with tokTj/wv in parallel on the gpsimd queue, so the first
        # projection chains and pair-0 scores start as early as possible.
        for kc in (0, 1):
            nc.sync.dma_start(tokT_sb[:, kc], tokT_d[:, kc])
        wslices = {(pr, w): load_w(pr, w) for pr in (0, 1) for w in ("q", "k")}
        for kc in range(2, NKC):
            nc.sync.dma_start(tokT_sb[:, kc], tokT_d[:, kc])
        for kc in range(NKC):
            nc.sync.dma_start(tokTj_sb[:, kc], tokTj_d[:, kc])
        for kc in range(NKC):
            nc.sync.dma_start(wv_sb[:, kc], wv_d[:, kc])

        def proj_chain(w_t, pr, which, t):
            """One 512-wide fp8-DR projection chain via a scores-pool slot."""
            dst, rhs_sb = (qT_sb, tokT_sb) if which == "q" else (kT_sb, tokTj_sb)
            ps = psS.tile([128, 512], F32, tag="sc", name=f"pj{which}{pr}_{t}")
            for c in range(NKC // 2):
                mm(
                    ps[:],
                    w_t[:, 2 * c : 2 * c + 2, :],
                    rhs_sb[:, 2 * c : 2 * c + 2, t * 512 : (t + 1) * 512],
                    start=(c == 0),
                    stop=(c == NKC // 2 - 1),
                    perf_mode=DR,
                )
            nc.vector.tensor_copy(out=dst[:, pr, t * 512 : (t + 1) * 512], in_=ps[:])

        def proj_v(ic):
            """V projection (fp8 DR) for one i-chunk via a scores-pool slot."""
            ps = psS.tile([128, 1024], F32, tag="sc", name=f"pjv{ic}")
            for c in range(NKC // 2):
                for nb in range(2):
                    mm(
                        ps[:, nb * 512 : (nb + 1) * 512],
                        tokT_sb[:, 2 * c : 2 * c + 2, ic * 128 : (ic + 1) * 128],
                        wv_sb[:, 2 * c : 2 * c + 2, nb * 512 : (nb + 1) * 512],
                        start=(c == 0),
                        stop=(c == NKC // 2 - 1),
                        perf_mode=DR,
                    )
            nc.vector.tensor_copy(
                out=v_sb[:, ic, :, 0:DV], in_=ps.rearrange("p (h v) -> p h v", h=H)
            )

        def exp_engine(pr, hh):
            # ACT always takes hh0; it also takes hh1 for pairs 0-1 while DVE
            # is busy with projection copies.
            return "act" if (hh == 0 or pr < 2) else "dve"

        def normalize_hh(pr, hh, acc):
            """multiT (x64, fp8) = heads / rowsum for one head. The PSUM->SBUF
            staging copy runs on ACT (idle at pair boundaries); rest DVE/DMA."""
            t, ko = pr // 2, pr % 2
            Ident = mybir.ActivationFunctionType.Identity
            hraw = pn_pool.tile([DV + 1, JW], F32, tag="hraw", name=f"hraw{pr}_{hh}")
            nc.scalar.activation(hraw[:], acc[hh][:], Ident)  # frees acc
            rs_dram = pdram.tile([1, JW], F32, tag="rsd", name=f"rsd{pr}_{hh}")
            nc.sync.dma_start(out=rs_dram[:], in_=hraw[DV : DV + 1, :])
            rec_in = pn_pool.tile([DV, JW], F32, tag="rin", name=f"rin{pr}_{hh}")
            nc.sync.dma_start(out=rec_in[:], in_=rs_dram.to_broadcast((DV, JW)))
            nc.vector.reciprocal_approx_fast(out=rec_in[:], in_=rec_in[:])
            if hh == 0:
                nc.vector.tensor_tensor(
                    multiT[t][0:DV, ko, :], hraw[0:DV, :], rec_in[:], mult
                )
            else:
                tmp64 = pn_pool.tile([DV, JW], FP8, tag="tmp64", name=f"tmp{pr}")
                nc.vector.tensor_tensor(tmp64[:], hraw[0:DV, :], rec_in[:], mult)
                nc.sync.dma_start(out=multiT[t][DV:128, ko, :], in_=tmp64[:])

        def attention(hooks_by_pr):
            """All pairs; attnV (fp8 DR over i-chunk pairs) lags the exp
            stream and is split by head: hh0 flushes on even steps, hh1 on
            odd steps, so every step has PE work covering the exp latency."""
            pending = None      # (pr, icp, eT, acc), flushed one hh at a time
            pending_hh = 0
            acc_by_pr = {}
            eT_cur = {}

            def flush_one():
                nonlocal pending, pending_hh
                if pending is None:
                    return
                ppr, picp, peT, pacc = pending
                hh = pending_hh
                for jb in range(2):
                    mm(
                        pacc[hh][:, jb * 512 : (jb + 1) * 512],
                        v_sb[:, 2 * picp : 2 * picp + 2, 2 * ppr + hh, :],
                        peT[hh][:, :, jb * 512 : (jb + 1) * 512],
                        start=(picp == 0),
                        stop=(picp == NIC // 2 - 1),
                        perf_mode=DR,
                    )
                if picp == NIC // 2 - 1:
                    normalize_hh(ppr, hh, pacc)
                if hh == 1:
                    pending = None
                pending_hh = 1 - pending_hh

            for pr in range(NPAIR):
                acc_by_pr[pr] = [
                    psAcc.tile([DV + 1, JW], F32, tag="acc", name=f"acc{pr}_{hh}")
                    for hh in range(2)
                ]
                hooks = hooks_by_pr.get(pr, {})
                for ic in range(NIC):
                    if ic % 2 == 0:
                        eT_cur = {
                            hh: pe_pool.tile(
                                [128, 2, JW], FP8, tag=f"eT{hh}",
                                name=f"eT{pr}_{ic // 2}_{hh}",
                            )
                            for hh in range(2)
                        }
                    ps_s = [
                        psS.tile([128, JW], F32, tag="sc", name=f"ps_s{pr}_{ic}_{hh}")
                        for hh in range(2)
                    ]
                    # scores^T, row-tiled pair (K=64 at partitions 0/64)
                    for hh in range(2):
                        for jb in range(2):
                            mm(
                                ps_s[hh][:, jb * 512 : (jb + 1) * 512],
                                qT_sb[hh * 64 : (hh + 1) * 64, pr, ic * 128 : (ic + 1) * 128],
                                kT_sb[hh * 64 : (hh + 1) * 64, pr, jb * 512 : (jb + 1) * 512],
                                start=True,
                                stop=True,
                            )
                    for hh in range(2):
                        slot = eT_cur[hh][:, ic % 2, :]
                        if exp_engine(pr, hh) == "act":
                            nc.scalar.activation(slot, ps_s[hh][:], Exp, scale=SCALE_EXP)
                        else:
                            nc.vector.tensor_scalar(
                                out=slot.bitcast(I8), in0=ps_s[hh][:],
                                scalar1=A_SCH, scalar2=B_SCH, op0=mult, op1=add,
                            )
                    flush_one()
                    if ic % 2 == 1:
                        assert pending is None
                        pending = (pr, (ic - 1) // 2, eT_cur, acc_by_pr[pr])
                    if ic in hooks:
                        for fn in hooks[ic]:
                            fn()
            flush_one()
            flush_one()

        # upfront: qT/kT for pairs 0-1 only; the rest rides inside the loop.
        for pr in (0, 1):
            for t in range(4):
                proj_chain(wslices[(pr, "q")], pr, "q", t)
            for t in range(2):
                proj_chain(wslices[(pr, "k")], pr, "k", t)

        hooks = {pr: {} for pr in range(NPAIR)}

        def add_hook(pr, ic, fn):
            hooks[pr].setdefault(ic, []).append(fn)

        # All hooks sit on odd steps, after the attnV flush, so their PSUM
        # slot (freed by an exp ~1 step earlier) never stalls the PE.
        # V projection: ALL 16 chunks inside pair 0, two per odd step —
        # chunk pair (2k, 2k+1) lands at step 2k+1, just ahead of the
        # attnV flush that consumes it at step 2k+2.
        for k in range(8):
            add_hook(0, 2 * k + 1, lambda k=k: (proj_v(2 * k), proj_v(2 * k + 1)))
        # Q/K projections: pairs 2-3 in pair 1 (doubled), pair p>=4 in pair
        # p-2 (one chain per odd step). Weight slices load one pair ahead.
        chains = [("q", 0), ("q", 1), ("q", 2), ("q", 3), ("k", 0), ("k", 1)]
        add_hook(0, 3, lambda: wslices.update({(2, "q"): load_w(2, "q"),
                                               (2, "k"): load_w(2, "k")}))
        add_hook(0, 9, lambda: wslices.update({(3, "q"): load_w(3, "q"),
                                               (3, "k"): load_w(3, "k")}))
        for i, (w, t) in enumerate(chains):
            add_hook(1, 2 * i + 1, lambda w=w, t=t: (
                proj_chain(wslices[(2, w)], 2, w, t),
                proj_chain(wslices[(3, w)], 3, w, t)))
        for p in range(4, NPAIR):
            host = p - 2
            add_hook(host - 1, 1, lambda p=p: wslices.update(
                {(p, "q"): load_w(p, "q"), (p, "k"): load_w(p, "k")}))
            for i, (w, t) in enumerate(chains):
                add_hook(host, 2 * i + 3, lambda p=p, w=w, t=t: proj_chain(
                    wslices[(p, w)], p, w, t))
        # phase C inputs arrive late so they don't compete with early DMAs
        def load_phase_c():
            nc.sync.dma_start(wo_sb[:], wo_d[:])
            nc.sync.dma_start(tokres_sb[:], tokres_d[:])
        add_hook(5, 14, load_phase_c)

        attention(hooks)
        psAcc.release()
        psS.release()

        # ---------------- Phase C: out-proj + residual + LayerNorm ----------------
        def outproj_chain(ps_o, jch, ts):
            for t in ts:
                if t == 3:
                    # split the last kc pair into plain fp8 matmuls so only
                    # the final one depends on pair 7's normalize
                    for ko in range(2):
                        for nb in range(2):
                            mm(
                                ps_o[:, nb * 512 : (nb + 1) * 512],
                                multiT[3][:, ko, jch * 128 : (jch + 1) * 128],
                                wo_sb[:, 6 + ko, nb * 512 : (nb + 1) * 512],
                                start=False,
                                stop=(ko == 1),
                            )
                    continue
                lhsT = multiT[t][:, :, jch * 128 : (jch + 1) * 128]
                for nb in range(2):
                    mm(
                        ps_o[:, nb * 512 : (nb + 1) * 512],
                        lhsT,
                        wo_sb[:, 2 * t : 2 * t + 2, nb * 512 : (nb + 1) * 512],
                        start=(t == 0),
                        stop=False,
                        perf_mode=DR,
                    )

        with (
            tc.tile_pool(name="pC", bufs=2) as pC,
            tc.tile_pool(name="pStats", bufs=8) as pStats,
            tc.tile_pool(name="psC", bufs=3, space="PSUM") as psC,
        ):
            # jch 0-1 start their kc-pairs 0-2 (pairs 0-5, long since
            # normalized) while pair 6/7's normalize still runs.
            early = [
                psC.tile([128, D], F32, tag="po", name=f"ps_o{jch}")
                for jch in range(2)
            ]
            for jch in range(2):
                outproj_chain(early[jch], jch, (0, 1, 2))
            for jch in range(NJCH):
                if jch < 2:
                    ps_o = early[jch]
                    outproj_chain(ps_o, jch, (3,))
                else:
                    ps_o = psC.tile([128, D], F32, tag="po", name=f"ps_o{jch}")
                    outproj_chain(ps_o, jch, (0, 1, 2, 3))
                # x = psum/2048 + residual, sum_t = rowsum(x), in one DVE pass
                x_sb = pC.tile([128, D], F32, tag="x", name=f"x{jch}")
                sum_t = pStats.tile([128, 1], F32, tag="sum", name=f"sum{jch}")
                nc.vector.scalar_tensor_tensor(
                    out=x_sb[:],
                    in0=ps_o[:],
                    scalar=CINV,
                    in1=tokres_sb[:, jch, :],
                    op0=mult,
                    op1=add,
                    accum_out=sum_t[:],
                )
                negmean = pStats.tile([128, 1], F32, tag="nm", name=f"nm{jch}")
                nc.vector.tensor_scalar_mul(negmean[:], sum_t[:], -1.0 / D)
                # ssq = sum((x-m)^2) on the otherwise-idle Scalar engine
                scrap = pC.tile([128, D], BF, tag="scrap", name=f"scrap{jch}")
                ssq = pStats.tile([128, 1], F32, tag="ssq", name=f"ssq{jch}")
                nc.scalar.activation(
                    scrap[:], x_sb[:], Square, bias=negmean[:], accum_out=ssq[:]
                )
                std_t = pStats.tile([128, 1], F32, tag="std", name=f"std{jch}")
                nc.scalar.activation(std_t[:], ssq[:], Sqrt, bias=eps_sb[:], scale=1.0 / D)
                rstd = pStats.tile([128, 1], F32, tag="rstd", name=f"rstd{jch}")
                nc.vector.reciprocal(rstd[:], std_t[:])
                rstd_nm = pStats.tile([128, 1], F32, tag="rnm", name=f"rnm{jch}")
                nc.vector.tensor_tensor(rstd_nm[:], negmean[:], rstd[:], mult)
                out_sb = pC.tile([128, D], F32, tag="out", name=f"out{jch}")
                nc.scalar.activation(
                    out_sb[:],
                    x_sb[:],
                    mybir.ActivationFunctionType.Identity,
                    bias=rstd_nm[:],
                    scale=rstd[:],
                )
                if apply_affine:
                    nc.gpsimd.tensor_tensor(out_sb[:], out_sb[:], gamma_sb[:], mult)
                    nc.gpsimd.tensor_tensor(out_sb[:], out_sb[:], beta_sb[:], add)
                (nc.sync if jch % 2 == 0 else nc.gpsimd).dma_start(out_d[:, jch], out_sb[:])

    nc.compile()
    return nc


def _prep_inputs(tokens, Wq, Wk, Wv, Wo, gamma, beta):
    """Host-side layout prep. Returns per-core input maps."""
    tokens = np.ascontiguousarray(np.asarray(tokens, dtype=np.float32))

    def rows128(a):  # [1024, N] -> [128, 8, N]
        return np.ascontiguousarray(
            a.reshape(NKC, 128, a.shape[-1]).transpose(1, 0, 2)
        )

    wq_all = rows128((np.asarray(Wq).transpose(1, 0, 2).reshape(D, H * DK) * WS).astype(F8))
    wk_all = rows128((np.asarray(Wk).transpose(1, 0, 2).reshape(D, H * DK) * WS).astype(F8))
    wv_all = rows128((np.asarray(Wv).transpose(1, 0, 2).reshape(D, H * DV) * WS).astype(F8))
    wo_all = rows128((np.asarray(Wo) * WS).astype(F8))
    gamma_bc = np.ascontiguousarray(
        np.broadcast_to(np.asarray(gamma, np.float32), (128, D))
    )
    beta_bc = np.ascontiguousarray(
        np.broadcast_to(np.asarray(beta, np.float32), (128, D))
    )

    tokT_by_b = []
    for b in range(B):
        tokT_by_b.append(rows128(tokens[b].T.astype(F8)))  # [128, 8, 2048] fp8

    in_maps = []
    for c in range(NCORES):
        b, jc = c // 2, c % 2
        tokT = tokT_by_b[b]
        tokTj = np.ascontiguousarray(tokT[:, :, jc * JW : (jc + 1) * JW])
        tokres = np.ascontiguousarray(
            tokens[b, jc * JW : (jc + 1) * JW]
            .reshape(NJCH, 128, D)
            .transpose(1, 0, 2)
        )
        in_maps.append(
            {
                "tokT": tokT,
                "tokTj": tokTj,
                "wq": wq_all,
                "wk": wk_all,
                "wv": wv_all,
                "wo": wo_all,
                "tokres": tokres,
                "gamma_bc": gamma_bc,
                "beta_bc": beta_bc,
            }
        )
    return in_maps


def run(inputs, trace=False, tmpdir=None):
    """Run on hardware; returns (output, BassKernelResults)."""
    from concourse.bass_utils import run_bass_kernel_spmd

    apply_affine = not (
        np.all(np.asarray(inputs["gamma"]) == 1.0)
        and np.all(np.asarray(inputs["beta"]) == 0.0)
    )
    key = ("nc", apply_affine)
    if key not in _CACHE:
        _CACHE[key] = _build_nc(apply_affine)
    nc = _CACHE[key]
    in_maps = _prep_inputs(**inputs)
    if not apply_affine:
        for m in in_maps:
            m.pop("gamma_bc"), m.pop("beta_bc")
    res = run_bass_kernel_spmd(
        nc, in_maps, core_ids=list(range(NCORES)), trace=trace, tmpdir=tmpdir
    )
    out = np.empty((B, S, D), np.float32)
    for c in range(NCORES):
        b, jc = c // 2, c % 2
        o = res.results[c]["out"]  # [128, 8, 1024]
        out[b, jc * JW : (jc + 1) * JW] = (
            o.transpose(1, 0, 2).reshape(JW, D)
        )
    return out, res


def kernel(tokens, Wq, Wk, Wv, Wo, gamma, beta):
    out, _ = run(
        dict(tokens=tokens, Wq=Wq, Wk=Wk, Wv=Wv, Wo=Wo, gamma=gamma, beta=beta)
    )
    return out
